# revision 47
# baseline (speedup 1.0000x reference)
"""Trainium2 Bass kernel for nn_MultiHeadAttention (N=2048, D=1024, H=16, causal).

Sharding: 16 heads split across 8 NeuronCores (2 heads/core, tensor-parallel
on the head dim).  Each core projects Q^T/K^T (its 128 head-dims x full
sequence) and V for its heads, computes causal attention in scores-transposed
layout ([nk, nq] blocks, softmax along the nk partition axis), applies its
128-row slice of Wo, and writes an fp16 partial [2048, 1024] output.  The
host sums the 8 partials and adds bo + bv@Wo_slice^T (the V bias is
mathematically a constant output row, so it never touches the device).

Structure: the proven v1 512-wide q-tile pipeline (per-tile q/k/v DMA
triplets, fine-grained score/exp emission with interleaved backend fillers)
with the arithmetic rework validated in v2:
  - PV flipped: probs blocks [128k, 128q] are the *stationary* matmul
    operand and the ones-augmented V [128k, 65] the moving one - 65
    free-columns per PV matmul instead of 128 (-7us PE), and the softmax
    denominator lands on the PV output's *partition* axis, collapsing
    normalization to a per-partition reciprocal + scale (the old chain
    burned ~16us of DVE + a PE broadcast per tile).
  - The normalized [q, dims] tile flips back to Wo's [dims, q] layout via
    a 53ns PE transpose (identity stationary) + a DVE copy.
  - V bias folded into the host-side output constant; Wq|Wk|bias packed
    into one fp8 tensor (bias bits bitcast in SBUF) - one front DMA.
  - Wo staging: both [128,512] halves f32->f16 on separate engines
    (Pool/DVE mid-flight, Act at the tail), one merged output DMA.
  - A dummy exp preloads the Act table during the DMA front.
  - Backend fillers emit at low scheduler priority: the exp train (the
    pacer at ~45us busy) always wins an engine when both are ready.
"""
import os
import sys

for _p in ("/opt/trn_rl_repo", "/root/.axon_site/_ro/trn_rl_repo"):
    if os.path.isdir(_p) and _p not in sys.path:
        sys.path.append(_p)

import numpy as np

import concourse.bass as bass
import concourse.mybir as mybir
from concourse import bacc
from concourse.bass_utils import run_bass_kernel_spmd
from concourse.tile import TileContext
from contextlib import ExitStack

N = 2048
D = 1024
NCORES = 8
DL = 128

F32 = mybir.dt.float32
F16 = mybir.dt.float16
F8 = mybir.dt.float8e4

# fp8 Q/K path: q, k, Wq, Wk in e4m3 (weights host-scaled x16 to clear the
# e4m3 denormal floor; bq, bk scaled to match; the extra 16*16 factor on the
# scores folds into the exp scale).  V path, probs, attnT, Wo stay f16 -
# every attempted fp8 extension of those paths measured >2e-2 end-to-end.
EXP_SCALE = 0.125 / 256.0


def build_nc(opts=None):
    nc = bacc.Bacc("TRN2", target_bir_lowering=False, debug=False,
                   num_devices=NCORES)

    # column-group-major: each 512-col tile load is one contiguous block
    qP = nc.dram_tensor("qP", [4, 128, 8, 512], F8, kind="ExternalInput")
    kP = nc.dram_tensor("kP", [4, 128, 8, 512], F8, kind="ExternalInput")
    vP = nc.dram_tensor("vP", [4, 128, 8, 512], F16, kind="ExternalInput")
    wqkP = nc.dram_tensor("wqkP", [128, 8, 2 * DL + 8], F8,
                          kind="ExternalInput")
    wvP = nc.dram_tensor("wvP", [128, 8, DL], F16, kind="ExternalInput")
    woP = nc.dram_tensor("woP", [DL, D], F16, kind="ExternalInput")
    out = nc.dram_tensor("out", [N, D], F16, kind="ExternalOutput")

    AF = mybir.ActivationFunctionType
    ALU = mybir.AluOpType
    DR = mybir.MatmulPerfMode.DoubleRow

    with TileContext(nc) as tc, ExitStack() as ctx:
        const = ctx.enter_context(tc.tile_pool(name="const", bufs=1))
        big = ctx.enter_context(tc.tile_pool(name="big", bufs=1))
        colp = ctx.enter_context(tc.tile_pool(name="colp", bufs=1))
        probsp = ctx.enter_context(tc.tile_pool(name="probsp", bufs=1))
        rcqp = ctx.enter_context(tc.tile_pool(name="rcqp", bufs=2))
        outp = ctx.enter_context(tc.tile_pool(name="outp", bufs=3))

        # ---- constants + input streaming, v1's per-tile (q, k, v)
        # triplet order so tile pipelines unlock progressively ----
        wqk = const.tile([128, 8, 2 * DL + 8], F8)
        nc.sync.dma_start(wqk[:], wqkP[:])
        wq = wqk[:, :, 0:DL]
        wk = wqk[:, :, DL:2 * DL]
        bqk_t = wqk[:, 0, 2 * DL:2 * DL + 8].bitcast(F32)
        qc, kc, vc = [], [], []

        def load_col(lst, name, dram, c, dt, eng):
            t = colp.tile([128, 8, 512], dt, name=f"{name}{c}")
            eng.dma_start(t[:], dram[c])
            lst.append(t)

        # all q/k first: the exp train (and the DMA-lane epoch guards the
        # tile framework pins on engine streams) then never reference a
        # v-era DMA; v/wv/wo feed backend work with ~20us of deadline slack
        for c in range(4):
            load_col(qc, "qc", qP, c, F8, nc.sync)
            load_col(kc, "kc", kP, c, F8, nc.scalar)
        wv = const.tile([128, 8, DL], F16)
        nc.sync.dma_start(wv[:], wvP[:])
        load_col(vc, "vc", vP, 0, F16, nc.sync)
        wo = const.tile([DL, D], F16)
        nc.sync.dma_start(wo[:], woP[:])
        load_col(vc, "vc", vP, 1, F16, nc.sync)
        load_col(vc, "vc", vP, 2, F16, nc.sync)
        load_col(vc, "vc", vP, 3, F16, nc.sync)

        ones64 = const.tile([1, 64], F16)
        nc.vector.memset(ones64[:], 1.0)
        ones512 = const.tile([1, 512], F16)
        nc.vector.memset(ones512[:], 1.0)
        # f16 identity for PE transposes: ones masked to the diagonal
        ident = const.tile([128, 128], F16)
        nc.vector.memset(ident[:], 1.0)
        nc.gpsimd.affine_select(
            out=ident[:], in_=ident[:], compare_op=ALU.is_ge,
            fill=0.0, base=0, pattern=[[1, 128]], channel_multiplier=-1)
        nc.gpsimd.affine_select(
            out=ident[:], in_=ident[:], compare_op=ALU.is_le,
            fill=0.0, base=0, pattern=[[1, 128]], channel_multiplier=-1)
        # dummy exp so the Act table load happens during the DMA front
        scratch = const.tile([1, 64], F16)
        nc.scalar.activation(scratch[:], ones64[:], AF.Exp, scale=1.0)

        # ---- persistent activations ----
        QT = big.tile([128, N], F16)
        KT = big.tile([128, N], F16)
        # Vaug[p_seq, head, seq_block, 65]: cols 0:64 projected V, col 64
        # ones (PV's moving operand; the ones column accumulates the
        # softmax denominator into PV-output column 64 for free)
        Vaug = big.tile([128, 2, 16, 65], F16)
        nc.vector.memset(Vaug[:, :, :, 64:65], 1.0)
        attnT = big.tile([128, N], F16)
        attnQ = [big.tile([128, 4, 128], F16, name=f"attnQ{t}")
                 for t in range(4)]

        # probs bookkeeping: probs_ref[(t, h)][b] = pb tile
        probs_ref = {}

        # PSUM budget (8 banks): scp 2x[128,2,512] scores/proj (block
        # PAIRS: one exp instruction covers two k-blocks - the ~185ns
        # fixed Act cost per exp was ~15us of the pacer engine's time),
        # wkp 2x[128,512] vproj/Wo/warmup, pvqp 2x[128,4,65] PV quads
        with tc.tile_pool(name="scp", bufs=2, space="PSUM") as scp, \
             tc.tile_pool(name="wkp", bufs=2, space="PSUM") as wkp, \
             tc.tile_pool(name="pvqp", bufs=2, space="PSUM") as pvqp:

            fillers = []
            LOWPRI = 1_000_000

            def emit_score_pair(t, h, bp):
                # k-blocks 2bp and 2bp+1 in one 2-bank tile, ONE exp.
                # Wedge blocks (b >= 4t) causally trimmed; a short even
                # block leaves junk PSUM cols under the (single) exp -
                # harmless, the pb junk region is never read.
                r0, r1 = 64 * h, 64 * (h + 1)
                sc = scp.tile([128, 2, 512], F32, name="sc")
                info = []
                for i, b in enumerate((2 * bp, 2 * bp + 1)):
                    wdg = b - 4 * t
                    off = 128 * wdg if wdg >= 0 else 0
                    width = 512 - off
                    info.append((b, wdg, width))
                    nc.tensor.matmul(
                        sc[:, i, 0:width],
                        KT[r0:r1, 128 * b:128 * (b + 1)],
                        QT[r0:r1, 512 * t + off:512 * (t + 1)],
                        start=True, stop=True, tile_position=(64 * h, 0))
                wtot = 512 + info[1][2]
                flat = sc.rearrange("p a b -> p (a b)")
                pb = probsp.tile([128, wtot], F16, name=f"pb{t}{h}{bp}")
                nc.scalar.activation(pb[:], flat[:, 0:wtot], AF.Exp,
                                     scale=EXP_SCALE)
                for i, (b, wdg, width) in enumerate(info):
                    if wdg >= 0:
                        nc.gpsimd.affine_select(
                            out=pb[:, 512 * i:512 * i + 128],
                            in_=pb[:, 512 * i:512 * i + 128],
                            compare_op=ALU.is_ge, fill=0.0,
                            base=0, pattern=[[1, 128]],
                            channel_multiplier=-1)
                    probs_ref[(t, h)][b] = (pb, 512 * i)

            def probs_slice(t, h, b, s):
                pb, base = probs_ref[(t, h)][b]
                wdg = max(0, b - 4 * t)
                c0 = base + 128 * (s - wdg)
                return pb[:, c0:c0 + 128]

            def emit_vproj(c, j):
                # seq block 4c+j -> region [:, 128j:128j+128] of a wkp tile
                if j == 0:
                    emit_vproj.vp = wkp.tile([128, 512], F32, name="wk")
                vp = emit_vproj.vp
                for u in range(8):
                    nc.tensor.matmul(
                        vp[:, 128 * j:128 * (j + 1)],
                        vc[c][:, u, 128 * j:128 * (j + 1)],
                        wv[:, u, :], start=(u == 0), stop=(u == 7))

            def emit_vcopy(c):
                vp = emit_vproj.vp.rearrange(
                    "p (bb g d) -> p g bb d", bb=4, g=2, d=64)
                nc.vector.tensor_scalar_mul(
                    Vaug[:, 0:2, 4 * c:4 * (c + 1), 0:64], vp[:], 1.0)

            pvq_ref = {}

            def emit_pv(t, h, s):
                # strip s: accumulate over k blocks b = 0..4t+s into
                # pvq[128q, 65] (probs stationary, Vaug moving: 65
                # free-cols per matmul; col 64 = denominator per q)
                if s == 0:
                    pvq_ref[(t, h)] = pvqp.tile([128, 4, 65], F32,
                                                name="pvq")
                pvq = pvq_ref[(t, h)]
                last = 4 * t + s
                for b in range(last + 1):
                    nc.tensor.matmul(
                        pvq[:, s, 0:65],
                        probs_slice(t, h, b, s),
                        Vaug[:, h, b, 0:65],
                        start=(b == 0), stop=(b == last))

            def emit_norm(t, h):
                pvq = pvq_ref[(t, h)]
                rcq = rcqp.tile([128, 4, 1], F32, name="rcq")
                nc.vector.reciprocal(rcq[:], pvq[:, :, 64:65])
                nc.vector.tensor_mul(
                    attnQ[t][:, :, 64 * h:64 * (h + 1)],
                    pvq[:, :, 0:64],
                    rcq.broadcast_to((128, 4, 64)))

            def emit_norm_strip(t, h, s):
                # per-strip normalize for the final tile's second head so
                # the post-exp tail pipelines strip-by-strip
                pvq = pvq_ref[(t, h)]
                rcq = rcqp.tile([128, 4, 1], F32, name="rcq")
                nc.vector.reciprocal(rcq[:, 0:1, :], pvq[:, s:s + 1, 64:65])
                nc.vector.tensor_mul(
                    attnQ[t][:, s:s + 1, 64 * h:64 * (h + 1)],
                    pvq[:, s:s + 1, 0:64],
                    rcq[:, 0:1, :].broadcast_to((128, 1, 64)))

            def emit_transpose(t, s, final=False):
                # PE transpose (53ns) + DVE copy flips [q, dims] ->
                # [dims, q]; PSUM slot from the pvq pool (freed by the
                # norm just before this).  The final tile's per-strip
                # chain uses the by-then-idle score pool instead (its h1
                # pvq quad stays live across all four strip-norms, so the
                # pvq pool can't rotate there).
                m = 4 * t + s
                pool = scp if final else pvqp
                tp = pool.tile([128, 128], F16,
                               name="sc" if final else "pvq")
                nc.tensor.transpose(tp[:], attnQ[t][:, s, :], ident[:])
                nc.vector.tensor_scalar_mul(
                    attnT[:, 128 * m:128 * (m + 1)], tp[:], 1.0)

            def emit_wo(t, s, stages):
                # both [128,512] halves through rotating wkp slots, each
                # staged f32->f16 on its own engine, one output DMA
                m = 4 * t + s
                ob = outp.tile([128, 1024], F16, name="ob")
                for u, stage in enumerate(stages):
                    wp = wkp.tile([128, 512], F32, name="wk")
                    nc.tensor.matmul(wp[:],
                                     attnT[:, 128 * m:128 * (m + 1)],
                                     wo[:, 512 * u:512 * (u + 1)],
                                     start=True, stop=True)
                    dst = ob[:, 512 * u:512 * (u + 1)]
                    if stage == "act":
                        nc.scalar.activation(dst, wp[:], AF.Copy)
                    elif stage == "pool":
                        nc.gpsimd.tensor_scalar_mul(dst, wp[:], 1.0)
                    else:
                        nc.vector.tensor_scalar_mul(dst, wp[:], 1.0)
                nc.sync.dma_start(out[128 * m:128 * (m + 1), :], ob[:])

            def emit_proj(t):
                # QT/KT columns [512t : 512t+512]; q and k share one
                # 2-bank tile; fp8 DoubleRow (256-wide contraction/pass);
                # each copy emitted right after its own half's matmuls
                ps = scp.tile([128, 2, 512], F32, name="sc")
                for half, (src_c, w, bcol, dst) in enumerate(
                        ((qc[t], wq, 0, QT), (kc[t], wk, 1, KT))):
                    for jj in range(4):
                        nc.tensor.matmul(
                            ps[:, half, :], w[:, 2 * jj:2 * jj + 2, :],
                            src_c[:, 2 * jj:2 * jj + 2, :],
                            start=(jj == 0), stop=(jj == 3),
                            perf_mode=DR)
                    nc.vector.tensor_scalar_add(
                        dst[:, 512 * t:512 * (t + 1)], ps[:, half, :],
                        bqk_t[:, bcol:bcol + 1])

            # PE warmup: keep the tensor engine busy through the initial
            # DMA front so the p-state ramp completes before the first
            # projection (ramp resets on idle; full clock after 3us)
            def warmup(n):
                for _ in range(n):
                    wu = wkp.tile([128, 512], F32, name="wk")
                    nc.tensor.matmul(wu[0:64, :], ones64[:], ones512[:],
                                     start=True, stop=True)

            groups_left = [sum(2 * t + 2 for t in range(4)) * 2]

            def pop_fillers():
                import math
                k = max(2, min(4, math.ceil(
                    len(fillers) / max(1, groups_left[0]))))
                # backend at low scheduler priority: the exp train always
                # wins an engine when both are ready
                with tc.high_priority(offset=-LOWPRI):
                    for _ in range(k):
                        if fillers:
                            fillers.pop(0)()

            def queue_pipe(t):
                fillers.extend(lambda j=j, c=t: emit_vproj(c, j)
                               for j in range(4))
                fillers.append(lambda c=t: emit_vcopy(c))
                fillers.extend(lambda s=s, t=t: emit_pv(t, 0, s)
                               for s in range(4))
                fillers.append(lambda t=t: emit_norm(t, 0))
                if t < 3:
                    fillers.extend(lambda s=s, t=t: emit_pv(t, 1, s)
                                   for s in range(4))
                    fillers.append(lambda t=t: emit_norm(t, 1))
                    for s in range(4):
                        fillers.append(lambda t=t, s=s:
                                       emit_transpose(t, s))
                        st = ("dve", "pool") if s % 2 == 0 else \
                             ("pool", "dve")
                        fillers.append(lambda t=t, s=s, st=st:
                                       emit_wo(t, s, st))
                else:
                    # final tile, second head: per-strip pipelining so the
                    # post-exp tail is one strip deep; staging on Act/DVE
                    # (idle once the exp train drains)
                    for s in range(4):
                        fillers.append(lambda s=s: emit_pv(3, 1, s))
                        fillers.append(lambda s=s: emit_norm_strip(3, 1, s))
                        fillers.append(lambda s=s:
                                       emit_transpose(3, s, final=True))
                        st = (("dve", "pool"), ("pool", "dve"),
                              ("act", "dve"), ("act", "dve"))[s]
                        fillers.append(lambda s=s, st=st:
                                       emit_wo(3, s, st))

            warmup(4)
            for t in range(4):
                emit_proj(t)
                if t == 0:
                    warmup(2)
                # pipes run one tile late: tile t-1's V projection + PV +
                # norm + transpose + Wo ride the bubbles of tile t's exp
                # train (v columns are deferred behind q/k in DMA order)
                if t >= 1:
                    queue_pipe(t - 1)
                for h in range(2):
                    probs_ref[(t, h)] = {}
                    for bp in range(2 * t + 2):
                        emit_score_pair(t, h, bp)
                        groups_left[0] -= 1
                        pop_fillers()
            queue_pipe(3)
            with tc.high_priority(offset=-LOWPRI):
                while fillers:
                    fillers.pop(0)()

    nc.compile()
    return nc


def make_in_maps(q, k, v, Wq, bq, Wk, bk, Wv, bv, Wo, bo):
    import ml_dtypes
    fp8 = ml_dtypes.float8_e4m3
    f32 = np.float32
    WSCALE = 16.0

    def pack_cols(x, dt):
        # [N, D] input -> x.T [D, N] -> [4, 128, 8, 512]: row (j*128+p),
        # cols [512c : 512c+512] at [c, p, j, :] (column-group-major so
        # each 512-col tile is one contiguous DRAM block)
        xt = np.ascontiguousarray(x.T.astype(f32))
        return np.ascontiguousarray(
            xt.reshape(8, 128, 4, 512).transpose(2, 1, 0, 3)).astype(dt)

    qPa, kPa = pack_cols(q, fp8), pack_cols(k, fp8)
    vPa = pack_cols(v, np.float16)
    WqT = Wq.T.astype(f32) * WSCALE
    WkT = Wk.T.astype(f32) * WSCALE
    WvT = Wv.T.astype(f32)
    WoT = Wo.T.astype(f32)

    def pack_w(WT, c, dt):
        # [D, DL] column slice -> [128, 8, DL]
        sl = np.ascontiguousarray(WT[:, DL * c:DL * (c + 1)])
        return np.ascontiguousarray(
            sl.reshape(8, 128, DL).transpose(1, 0, 2)).astype(dt)

    in_maps = []
    for c in range(NCORES):
        d0 = DL * c
        # wq | wk | bqk(f32, bit-packed into 8 fp8 lanes of j=0)
        wqk = np.zeros((128, 8, 2 * DL + 8), dtype=np.uint8)
        wqk[:, :, 0:DL] = pack_w(WqT, c, fp8).view(np.uint8)
        wqk[:, :, DL:2 * DL] = pack_w(WkT, c, fp8).view(np.uint8)
        bqk = np.ascontiguousarray(
            np.stack([bq[d0:d0 + DL] * WSCALE,
                      bk[d0:d0 + DL] * WSCALE], axis=1)).astype(f32)
        wqk[:, 0, 2 * DL:2 * DL + 8] = bqk.view(np.uint8).reshape(128, 8)
        in_maps.append({
            "qP": qPa, "kP": kPa, "vP": vPa,
            "wqkP": wqk.view(fp8),
            "wvP": pack_w(WvT, c, np.float16),
            "woP": np.ascontiguousarray(WoT[d0:d0 + DL, :]).astype(np.float16),
        })
    return in_maps


_NC_CACHE = None


def _get_nc():
    global _NC_CACHE
    if _NC_CACHE is None:
        _NC_CACHE = build_nc()
    return _NC_CACHE


def kernel(q, k, v, Wq, bq, Wk, bk, Wv, bv, Wo, bo):
    """Full-input / full-output entry point (harness contract)."""
    q, k, v = np.asarray(q), np.asarray(k), np.asarray(v)
    Wq, bq, Wk, bk = np.asarray(Wq), np.asarray(bq), np.asarray(Wk), np.asarray(bk)
    Wv, bv, Wo, bo = np.asarray(Wv), np.asarray(bv), np.asarray(Wo), np.asarray(bo)
    nc = _get_nc()
    in_maps = make_in_maps(q, k, v, Wq, bq, Wk, bk, Wv, bv, Wo, bo)
    res = run_bass_kernel_spmd(nc, in_maps, list(range(NCORES)))
    acc = res.results[0]["out"].astype(np.float64)
    for c in range(1, NCORES):
        acc += res.results[c]["out"]
    # V bias folded host-side: concat rows carry +bv per head-dim, so the
    # device-side output is short exactly bv @ Wo^T (a constant row)
    acc += (bv.astype(np.float64) @ Wo.T.astype(np.float64))
    acc += bo.astype(np.float64)
    return acc.astype(np.float32)


# revision 49
# speedup vs baseline: 1.0319x; 1.0319x over previous
"""Trainium2 Bass kernel for nn_MultiHeadAttention (N=2048, D=1024, H=16, causal).

Sharding: 16 heads split across 8 NeuronCores (2 heads/core, tensor-parallel
on the head dim).  Each core projects Q^T/K^T (its 128 head-dims x full
sequence) and V for its heads, computes causal attention in scores-transposed
layout ([nk, nq] blocks, softmax along the nk partition axis), applies its
128-row slice of Wo, and writes an fp16 partial [2048, 1024] output.  The
host sums the 8 partials and adds bo + bv@Wo_slice^T (the V bias is
mathematically a constant output row, so it never touches the device).

Structure: the proven v1 512-wide q-tile pipeline (per-tile q/k/v DMA
triplets, fine-grained score/exp emission with interleaved backend fillers)
with the arithmetic rework validated in v2:
  - PV flipped: probs blocks [128k, 128q] are the *stationary* matmul
    operand and the ones-augmented V [128k, 65] the moving one - 65
    free-columns per PV matmul instead of 128 (-7us PE), and the softmax
    denominator lands on the PV output's *partition* axis, collapsing
    normalization to a per-partition reciprocal + scale (the old chain
    burned ~16us of DVE + a PE broadcast per tile).
  - The normalized [q, dims] tile flips back to Wo's [dims, q] layout via
    a 53ns PE transpose (identity stationary) + a DVE copy.
  - V bias folded into the host-side output constant; Wq|Wk|bias packed
    into one fp8 tensor (bias bits bitcast in SBUF) - one front DMA.
  - Wo staging: both [128,512] halves f32->f16 on separate engines
    (Pool/DVE mid-flight, Act at the tail), one merged output DMA.
  - A dummy exp preloads the Act table during the DMA front.
  - Backend fillers emit at low scheduler priority: the exp train (the
    pacer at ~45us busy) always wins an engine when both are ready.
"""
import os
import sys

for _p in ("/opt/trn_rl_repo", "/root/.axon_site/_ro/trn_rl_repo"):
    if os.path.isdir(_p) and _p not in sys.path:
        sys.path.append(_p)

import numpy as np

import concourse.bass as bass
import concourse.mybir as mybir
from concourse import bacc
from concourse.bass_utils import run_bass_kernel_spmd
from concourse.tile import TileContext
from contextlib import ExitStack

N = 2048
D = 1024
NCORES = 8
DL = 128

F32 = mybir.dt.float32
F16 = mybir.dt.float16
F8 = mybir.dt.float8e4

# fp8 Q/K path: q, k, Wq, Wk in e4m3 (weights host-scaled x16 to clear the
# e4m3 denormal floor; bq, bk scaled to match; the extra 16*16 factor on the
# scores folds into the exp scale).  V path, probs, attnT, Wo stay f16 -
# every attempted fp8 extension of those paths measured >2e-2 end-to-end.
EXP_SCALE = 0.125 / 256.0


def build_nc(opts=None):
    nc = bacc.Bacc("TRN2", target_bir_lowering=False, debug=False,
                   num_devices=NCORES)

    # column-group-major: each 512-col tile load is one contiguous block
    qP = nc.dram_tensor("qP", [4, 128, 8, 512], F8, kind="ExternalInput")
    kP = nc.dram_tensor("kP", [4, 128, 8, 512], F8, kind="ExternalInput")
    vP = nc.dram_tensor("vP", [4, 128, 8, 512], F16, kind="ExternalInput")
    wqkP = nc.dram_tensor("wqkP", [128, 8, 2 * DL + 8], F8,
                          kind="ExternalInput")
    wvP = nc.dram_tensor("wvP", [128, 8, DL], F16, kind="ExternalInput")
    woP = nc.dram_tensor("woP", [DL, D], F16, kind="ExternalInput")
    out = nc.dram_tensor("out", [N, D], F16, kind="ExternalOutput")

    AF = mybir.ActivationFunctionType
    ALU = mybir.AluOpType
    DR = mybir.MatmulPerfMode.DoubleRow

    with TileContext(nc) as tc, ExitStack() as ctx:
        const = ctx.enter_context(tc.tile_pool(name="const", bufs=1))
        big = ctx.enter_context(tc.tile_pool(name="big", bufs=1))
        colp = ctx.enter_context(tc.tile_pool(name="colp", bufs=1))
        probsp = ctx.enter_context(tc.tile_pool(name="probsp", bufs=1))
        rcqp = ctx.enter_context(tc.tile_pool(name="rcqp", bufs=2))
        outp = ctx.enter_context(tc.tile_pool(name="outp", bufs=3))

        # ---- constants + input streaming, v1's per-tile (q, k, v)
        # triplet order so tile pipelines unlock progressively ----
        wqk = const.tile([128, 8, 2 * DL + 8], F8)
        nc.sync.dma_start(wqk[:], wqkP[:])
        wq = wqk[:, :, 0:DL]
        wk = wqk[:, :, DL:2 * DL]
        bqk_t = wqk[:, 0, 2 * DL:2 * DL + 8].bitcast(F32)
        qc, kc, vc = [], [], []

        def load_col(lst, name, dram, c, dt, eng):
            t = colp.tile([128, 8, 512], dt, name=f"{name}{c}")
            eng.dma_start(t[:], dram[c])
            lst.append(t)

        # all q/k first: the exp train (and the DMA-lane epoch guards the
        # tile framework pins on engine streams) then never reference a
        # v-era DMA; v/wv/wo feed backend work with ~20us of deadline slack
        for c in range(4):
            load_col(qc, "qc", qP, c, F8, nc.sync)
            load_col(kc, "kc", kP, c, F8, nc.scalar)
        wv = const.tile([128, 8, DL], F16)
        nc.sync.dma_start(wv[:], wvP[:])
        load_col(vc, "vc", vP, 0, F16, nc.sync)
        wo = const.tile([DL, D], F16)
        nc.sync.dma_start(wo[:], woP[:])
        load_col(vc, "vc", vP, 1, F16, nc.sync)
        load_col(vc, "vc", vP, 2, F16, nc.sync)
        load_col(vc, "vc", vP, 3, F16, nc.sync)

        ones64 = const.tile([1, 64], F16)
        nc.vector.memset(ones64[:], 1.0)
        ones512 = const.tile([1, 512], F16)
        nc.vector.memset(ones512[:], 1.0)
        # f16 identity for PE transposes: ones masked to the diagonal
        ident = const.tile([128, 128], F16)
        nc.vector.memset(ident[:], 1.0)
        nc.gpsimd.affine_select(
            out=ident[:], in_=ident[:], compare_op=ALU.is_ge,
            fill=0.0, base=0, pattern=[[1, 128]], channel_multiplier=-1)
        nc.gpsimd.affine_select(
            out=ident[:], in_=ident[:], compare_op=ALU.is_le,
            fill=0.0, base=0, pattern=[[1, 128]], channel_multiplier=-1)
        # dummy exp so the Act table load happens during the DMA front
        scratch = const.tile([1, 64], F16)
        nc.scalar.activation(scratch[:], ones64[:], AF.Exp, scale=1.0)

        # ---- persistent activations ----
        QT = big.tile([128, N], F16)
        KT = big.tile([128, N], F16)
        # Vaug[p_seq, head, seq_block, 65]: cols 0:64 projected V, col 64
        # ones (PV's moving operand; the ones column accumulates the
        # softmax denominator into PV-output column 64 for free)
        Vaug = big.tile([128, 2, 16, 65], F16)
        nc.vector.memset(Vaug[:, :, :, 64:65], 1.0)
        attnT = big.tile([128, N], F16)
        attnQ = [big.tile([128, 4, 128], F16, name=f"attnQ{t}")
                 for t in range(4)]

        # probs bookkeeping: probs_ref[(t, h)][b] = pb tile
        probs_ref = {}

        # PSUM budget (8 banks): scp 2x[128,2,512] scores/proj (block
        # PAIRS: one exp instruction covers two k-blocks - the ~185ns
        # fixed Act cost per exp was ~15us of the pacer engine's time),
        # wkp 2x[128,512] vproj/Wo/warmup, pvqp 2x[128,4,65] PV quads
        with tc.tile_pool(name="scp", bufs=2, space="PSUM") as scp, \
             tc.tile_pool(name="wkp", bufs=2, space="PSUM") as wkp, \
             tc.tile_pool(name="pvqp", bufs=2, space="PSUM") as pvqp:

            fillers = []
            LOWPRI = 1_000_000

            def emit_score_pair(t, h, bp):
                # k-blocks 2bp and 2bp+1 in one 2-bank tile, ONE exp.
                # Wedge blocks (b >= 4t) causally trimmed; a short even
                # block leaves junk PSUM cols under the (single) exp -
                # harmless, the pb junk region is never read.
                r0, r1 = 64 * h, 64 * (h + 1)
                sc = scp.tile([128, 2, 512], F32, name="sc")
                info = []
                for i, b in enumerate((2 * bp, 2 * bp + 1)):
                    wdg = b - 4 * t
                    off = 128 * wdg if wdg >= 0 else 0
                    width = 512 - off
                    info.append((b, wdg, width))
                    nc.tensor.matmul(
                        sc[:, i, 0:width],
                        KT[r0:r1, 128 * b:128 * (b + 1)],
                        QT[r0:r1, 512 * t + off:512 * (t + 1)],
                        start=True, stop=True, tile_position=(64 * h, 0))
                wtot = 512 + info[1][2]
                flat = sc.rearrange("p a b -> p (a b)")
                pb = probsp.tile([128, wtot], F16, name=f"pb{t}{h}{bp}")
                nc.scalar.activation(pb[:], flat[:, 0:wtot], AF.Exp,
                                     scale=EXP_SCALE)
                for i, (b, wdg, width) in enumerate(info):
                    if wdg >= 0:
                        nc.gpsimd.affine_select(
                            out=pb[:, 512 * i:512 * i + 128],
                            in_=pb[:, 512 * i:512 * i + 128],
                            compare_op=ALU.is_ge, fill=0.0,
                            base=0, pattern=[[1, 128]],
                            channel_multiplier=-1)
                    probs_ref[(t, h)][b] = (pb, 512 * i)

            def probs_slice(t, h, b, s):
                pb, base = probs_ref[(t, h)][b]
                wdg = max(0, b - 4 * t)
                c0 = base + 128 * (s - wdg)
                return pb[:, c0:c0 + 128]

            def emit_vproj(c, j):
                # seq block 4c+j -> region [:, 128j:128j+128] of a wkp tile
                if j == 0:
                    emit_vproj.vp = wkp.tile([128, 512], F32, name="wk")
                vp = emit_vproj.vp
                for u in range(8):
                    nc.tensor.matmul(
                        vp[:, 128 * j:128 * (j + 1)],
                        vc[c][:, u, 128 * j:128 * (j + 1)],
                        wv[:, u, :], start=(u == 0), stop=(u == 7))

            def emit_vcopy(c):
                vp = emit_vproj.vp.rearrange(
                    "p (bb g d) -> p g bb d", bb=4, g=2, d=64)
                nc.vector.tensor_scalar_mul(
                    Vaug[:, 0:2, 4 * c:4 * (c + 1), 0:64], vp[:], 1.0)

            pvq_ref = {}

            def emit_pv(t, h, s):
                # strip s: accumulate over k blocks b = 0..4t+s into
                # pvq[128q, 65] (probs stationary, Vaug moving: 65
                # free-cols per matmul; col 64 = denominator per q)
                if s == 0:
                    pvq_ref[(t, h)] = pvqp.tile([128, 4, 65], F32,
                                                name="pvq")
                pvq = pvq_ref[(t, h)]
                last = 4 * t + s
                for b in range(last + 1):
                    nc.tensor.matmul(
                        pvq[:, s, 0:65],
                        probs_slice(t, h, b, s),
                        Vaug[:, h, b, 0:65],
                        start=(b == 0), stop=(b == last))

            def emit_norm(t, h):
                pvq = pvq_ref[(t, h)]
                rcq = rcqp.tile([128, 4, 1], F32, name="rcq")
                nc.vector.reciprocal(rcq[:], pvq[:, :, 64:65])
                nc.vector.tensor_mul(
                    attnQ[t][:, :, 64 * h:64 * (h + 1)],
                    pvq[:, :, 0:64],
                    rcq.broadcast_to((128, 4, 64)))

            def emit_norm_strip(t, h, s):
                # per-strip normalize for the final tile's second head so
                # the post-exp tail pipelines strip-by-strip
                pvq = pvq_ref[(t, h)]
                rcq = rcqp.tile([128, 4, 1], F32, name="rcq")
                nc.vector.reciprocal(rcq[:, 0:1, :], pvq[:, s:s + 1, 64:65])
                nc.vector.tensor_mul(
                    attnQ[t][:, s:s + 1, 64 * h:64 * (h + 1)],
                    pvq[:, s:s + 1, 0:64],
                    rcq[:, 0:1, :].broadcast_to((128, 1, 64)))

            def emit_transpose(t, s, final=False):
                # PE transpose (53ns) + DVE copy flips [q, dims] ->
                # [dims, q]; PSUM slot from the pvq pool (freed by the
                # norm just before this).  The final tile's per-strip
                # chain uses the by-then-idle score pool instead (its h1
                # pvq quad stays live across all four strip-norms, so the
                # pvq pool can't rotate there).
                m = 4 * t + s
                pool = scp if final else pvqp
                tp = pool.tile([128, 128], F16,
                               name="sc" if final else "pvq")
                nc.tensor.transpose(tp[:], attnQ[t][:, s, :], ident[:])
                nc.vector.tensor_scalar_mul(
                    attnT[:, 128 * m:128 * (m + 1)], tp[:], 1.0)

            def emit_wo(t, s, stages):
                # both [128,512] halves through rotating wkp slots, each
                # staged f32->f16 on its own engine, one output DMA
                m = 4 * t + s
                ob = outp.tile([128, 1024], F16, name="ob")
                for u, stage in enumerate(stages):
                    wp = wkp.tile([128, 512], F32, name="wk")
                    nc.tensor.matmul(wp[:],
                                     attnT[:, 128 * m:128 * (m + 1)],
                                     wo[:, 512 * u:512 * (u + 1)],
                                     start=True, stop=True)
                    dst = ob[:, 512 * u:512 * (u + 1)]
                    if stage == "act":
                        nc.scalar.activation(dst, wp[:], AF.Copy)
                    elif stage == "pool":
                        nc.gpsimd.tensor_scalar_mul(dst, wp[:], 1.0)
                    else:
                        nc.vector.tensor_scalar_mul(dst, wp[:], 1.0)
                nc.sync.dma_start(out[128 * m:128 * (m + 1), :], ob[:])

            def emit_proj(t):
                # QT/KT columns [512t : 512t+512]; q and k share one
                # 2-bank tile; fp8 DoubleRow (256-wide contraction/pass);
                # each copy emitted right after its own half's matmuls
                ps = scp.tile([128, 2, 512], F32, name="sc")
                for half, (src_c, w, bcol, dst) in enumerate(
                        ((qc[t], wq, 0, QT), (kc[t], wk, 1, KT))):
                    for jj in range(4):
                        nc.tensor.matmul(
                            ps[:, half, :], w[:, 2 * jj:2 * jj + 2, :],
                            src_c[:, 2 * jj:2 * jj + 2, :],
                            start=(jj == 0), stop=(jj == 3),
                            perf_mode=DR)
                    nc.vector.tensor_scalar_add(
                        dst[:, 512 * t:512 * (t + 1)], ps[:, half, :],
                        bqk_t[:, bcol:bcol + 1])

            # PE warmup: keep the tensor engine busy through the initial
            # DMA front so the p-state ramp completes before the first
            # projection (ramp resets on idle; full clock after 3us)
            def warmup(n):
                for _ in range(n):
                    wu = wkp.tile([128, 512], F32, name="wk")
                    nc.tensor.matmul(wu[0:64, :], ones64[:], ones512[:],
                                     start=True, stop=True)

            groups_left = [sum(2 * t + 2 for t in range(4)) * 2]

            # real arrival times (us) of the v column tiles under the
            # qk-first DMA order: the scheduler's virtual DMA model is
            # optimistic, so V-gated backend work is pinned at these
            # timestamps (tile_wait_until) or it gets scheduled at the
            # head of engine queues and blocks the real critical path
            VC_US = (18.5, 22.0, 25.0, 28.0)

            def pop_fillers():
                import math
                k = max(2, min(4, math.ceil(
                    len(fillers) / max(1, groups_left[0]))))
                # backend at low scheduler priority: the exp train always
                # wins an engine when both are ready
                with tc.high_priority(offset=-LOWPRI):
                    for _ in range(k):
                        if fillers:
                            gate_us, fn = fillers.pop(0)
                            with tc.tile_wait_until(gate_us / 1000.0,
                                                    enable=gate_us > 0):
                                fn()

            def queue_pipe(t):
                gate = VC_US[t]
                fillers.extend((gate, lambda j=j, c=t: emit_vproj(c, j))
                               for j in range(4))
                fillers.append((gate, lambda c=t: emit_vcopy(c)))
                fillers.extend((0, lambda s=s, t=t: emit_pv(t, 0, s))
                               for s in range(4))
                fillers.append((0, lambda t=t: emit_norm(t, 0)))
                if t < 3:
                    fillers.extend((0, lambda s=s, t=t: emit_pv(t, 1, s))
                                   for s in range(4))
                    fillers.append((0, lambda t=t: emit_norm(t, 1)))
                    for s in range(4):
                        fillers.append((0, lambda t=t, s=s:
                                        emit_transpose(t, s)))
                        st = ("dve", "pool") if s % 2 == 0 else \
                             ("pool", "dve")
                        fillers.append((0, lambda t=t, s=s, st=st:
                                        emit_wo(t, s, st)))
                else:
                    # final tile, second head: per-strip pipelining so the
                    # post-exp tail is one strip deep; staging on Act/DVE
                    # (idle once the exp train drains)
                    for s in range(4):
                        fillers.append((0, lambda s=s: emit_pv(3, 1, s)))
                        fillers.append((0, lambda s=s:
                                        emit_norm_strip(3, 1, s)))
                        fillers.append((0, lambda s=s:
                                        emit_transpose(3, s, final=True)))
                        st = (("dve", "pool"), ("pool", "dve"),
                              ("act", "dve"), ("act", "dve"))[s]
                        fillers.append((0, lambda s=s, st=st:
                                        emit_wo(3, s, st)))

            warmup(4)
            for t in range(4):
                emit_proj(t)
                if t == 0:
                    warmup(2)
                # pipes run one tile late: tile t-1's V projection + PV +
                # norm + transpose + Wo ride the bubbles of tile t's exp
                # train (v columns are deferred behind q/k in DMA order)
                if t >= 1:
                    queue_pipe(t - 1)
                for h in range(2):
                    probs_ref[(t, h)] = {}
                    for bp in range(2 * t + 2):
                        emit_score_pair(t, h, bp)
                        groups_left[0] -= 1
                        pop_fillers()
            queue_pipe(3)
            with tc.high_priority(offset=-LOWPRI):
                while fillers:
                    gate_us, fn = fillers.pop(0)
                    with tc.tile_wait_until(gate_us / 1000.0,
                                            enable=gate_us > 0):
                        fn()

    nc.compile()
    return nc


def make_in_maps(q, k, v, Wq, bq, Wk, bk, Wv, bv, Wo, bo):
    import ml_dtypes
    fp8 = ml_dtypes.float8_e4m3
    f32 = np.float32
    WSCALE = 16.0

    def pack_cols(x, dt):
        # [N, D] input -> x.T [D, N] -> [4, 128, 8, 512]: row (j*128+p),
        # cols [512c : 512c+512] at [c, p, j, :] (column-group-major so
        # each 512-col tile is one contiguous DRAM block)
        xt = np.ascontiguousarray(x.T.astype(f32))
        return np.ascontiguousarray(
            xt.reshape(8, 128, 4, 512).transpose(2, 1, 0, 3)).astype(dt)

    qPa, kPa = pack_cols(q, fp8), pack_cols(k, fp8)
    vPa = pack_cols(v, np.float16)
    WqT = Wq.T.astype(f32) * WSCALE
    WkT = Wk.T.astype(f32) * WSCALE
    WvT = Wv.T.astype(f32)
    WoT = Wo.T.astype(f32)

    def pack_w(WT, c, dt):
        # [D, DL] column slice -> [128, 8, DL]
        sl = np.ascontiguousarray(WT[:, DL * c:DL * (c + 1)])
        return np.ascontiguousarray(
            sl.reshape(8, 128, DL).transpose(1, 0, 2)).astype(dt)

    in_maps = []
    for c in range(NCORES):
        d0 = DL * c
        # wq | wk | bqk(f32, bit-packed into 8 fp8 lanes of j=0)
        wqk = np.zeros((128, 8, 2 * DL + 8), dtype=np.uint8)
        wqk[:, :, 0:DL] = pack_w(WqT, c, fp8).view(np.uint8)
        wqk[:, :, DL:2 * DL] = pack_w(WkT, c, fp8).view(np.uint8)
        bqk = np.ascontiguousarray(
            np.stack([bq[d0:d0 + DL] * WSCALE,
                      bk[d0:d0 + DL] * WSCALE], axis=1)).astype(f32)
        wqk[:, 0, 2 * DL:2 * DL + 8] = bqk.view(np.uint8).reshape(128, 8)
        in_maps.append({
            "qP": qPa, "kP": kPa, "vP": vPa,
            "wqkP": wqk.view(fp8),
            "wvP": pack_w(WvT, c, np.float16),
            "woP": np.ascontiguousarray(WoT[d0:d0 + DL, :]).astype(np.float16),
        })
    return in_maps


_NC_CACHE = None


def _get_nc():
    global _NC_CACHE
    if _NC_CACHE is None:
        _NC_CACHE = build_nc()
    return _NC_CACHE


def kernel(q, k, v, Wq, bq, Wk, bk, Wv, bv, Wo, bo):
    """Full-input / full-output entry point (harness contract)."""
    q, k, v = np.asarray(q), np.asarray(k), np.asarray(v)
    Wq, bq, Wk, bk = np.asarray(Wq), np.asarray(bq), np.asarray(Wk), np.asarray(bk)
    Wv, bv, Wo, bo = np.asarray(Wv), np.asarray(bv), np.asarray(Wo), np.asarray(bo)
    nc = _get_nc()
    in_maps = make_in_maps(q, k, v, Wq, bq, Wk, bk, Wv, bv, Wo, bo)
    res = run_bass_kernel_spmd(nc, in_maps, list(range(NCORES)))
    acc = res.results[0]["out"].astype(np.float64)
    for c in range(1, NCORES):
        acc += res.results[c]["out"]
    # V bias folded host-side: concat rows carry +bv per head-dim, so the
    # device-side output is short exactly bv @ Wo^T (a constant row)
    acc += (bv.astype(np.float64) @ Wo.T.astype(np.float64))
    acc += bo.astype(np.float64)
    return acc.astype(np.float32)
